# revision 32
# baseline (speedup 1.0000x reference)
"""Trainium2 Bass kernel for one transformer block (B=2, T=2048, C=768, H=12,
inner=3072, fp32, causal, post-norm residual).

Sharding: 8 cores, token-interleaved. Core c handles batch c//4, tokens
p::4 (p = c%4) of that batch — every core runs the IDENTICAL program
(SPMD), causality is entirely data-driven via per-core mask tensors.

Layout: all activations transposed [C, tokens] so matmul contractions land
on partitions. K/V are (redundantly) computed for the full 2048 tokens on
every core; attention runs as scores^T -> exp -> [V|ones] matmul which
accumulates ctx^T and the softmax denominator in one pass (no max
subtraction needed: scores are ~N(0,1), |s|max << fp32 exp range).
Partition-dim reductions (softmax sums, layernorm stats) use ones-vector
matmuls on the PE; partition broadcasts use K=1 outer-product matmuls.

Matmuls run in float32r (tf32-like, full PE rate at N>=256); everything
else fp32.
"""

import sys

if "/opt/trn_rl_repo" not in sys.path:
    sys.path.insert(0, "/opt/trn_rl_repo")

import numpy as np
import ml_dtypes

import concourse.bacc as bacc
import concourse.mybir as mybir
import concourse.tile as tile
from concourse.bass_utils import run_bass_kernel_spmd

F32 = mybir.dt.float32
F32R = mybir.dt.float32r
BF16 = mybir.dt.bfloat16
ACTF = mybir.ActivationFunctionType

B, T, C = 2, 2048, 768
H, DH = 12, 64
IN = 3072
CC = C // 128          # 6 channel chunks
TBN = T // 512         # 4 token blocks of full seq
TQ = 512               # tokens per core
KCN = T // 128         # 16 k-chunks
ICN = IN // 128        # 24 inner chunks
HG = 2                 # head groups
HPG = H // HG          # 6 heads per group
EPS = 1e-4
SCALE = 1.0 / np.sqrt(DH)

# param pack order in "prk" [128, CC, 8]
P_BQ, P_BK, P_BO, P_B2, P_L1S, P_L1B, P_L2S, P_L2B = range(8)

DEBUG_TAPS = False


def _build_nc():
    nc = bacc.Bacc("TRN2", target_bir_lowering=False, debug=False,
                   enable_asserts=False, num_devices=8)
    d = {}
    d["xt"] = nc.dram_tensor("xt", [C, T], F32R, kind="ExternalInput").ap()
    d["xtq"] = nc.dram_tensor("xtq", [C, TQ], F32R, kind="ExternalInput").ap()
    for w, sh in (("wq", [C, C]), ("wk", [C, C]), ("wv", [C, C]),
                  ("wo", [C, C]), ("w1", [C, IN]), ("w2", [IN, C])):
        d[w] = nc.dram_tensor(w, sh, F32R, kind="ExternalInput").ap()
    d["prk"] = nc.dram_tensor("prk", [128, CC, 8], F32, kind="ExternalInput").ap()
    d["b1p"] = nc.dram_tensor("b1p", [128, ICN], F32, kind="ExternalInput").ap()
    d["bvb"] = nc.dram_tensor("bvb", [128, C], F32, kind="ExternalInput").ap()
    d["msk"] = nc.dram_tensor("msk", [KCN // 2, 128, 2 * TQ], BF16, kind="ExternalInput").ap()
    d["ones"] = nc.dram_tensor("ones", [128, 128], F32R, kind="ExternalInput").ap()
    d["zer"] = nc.dram_tensor("zer", [DH, TQ], F32R, kind="ExternalInput").ap()
    d["outT"] = nc.dram_tensor("outT", [C, TQ], F32, kind="ExternalOutput").ap()
    taps = {}
    if DEBUG_TAPS:
        taps["qT"] = nc.dram_tensor("qT", [C, TQ], F32, kind="ExternalOutput").ap()
        taps["kT"] = nc.dram_tensor("kT", [C, T], F32, kind="ExternalOutput").ap()
        taps["v"] = nc.dram_tensor("v", [T, C], F32, kind="ExternalOutput").ap()
        taps["ctxT"] = nc.dram_tensor("ctxT", [C, TQ], F32, kind="ExternalOutput").ap()
        taps["hT"] = nc.dram_tensor("hT", [C, TQ], F32, kind="ExternalOutput").ap()

    with tile.TileContext(nc) as tc:
        _emit(nc, tc, d, taps)
    nc.finalize()
    return nc


def _tap(nc, tc, dst, tiles, width):
    with tc.tile_pool(name="tapp", bufs=2, side="right") as tp:
        for i, src in enumerate(tiles):
            t = tp.tile([128, width], F32, name="tapt", tag="t")
            nc.vector.tensor_copy(t[:], src)
            nc.sync.dma_start(out=dst[i * 128:(i + 1) * 128, :], in_=t[:])


def _ln_stats(nc, pool, ps_sum, ps_sq, tagp):
    """ACT-copy psum stats rows to SBUF (inside the psum pool's scope)."""
    n = float(C)
    mean = pool.tile([1, TQ], F32R, name="ln_mean", tag=tagp + "mean")
    nc.scalar.activation(mean[:], ps_sum[:], ACTF.Copy, scale=1.0 / n)
    ex2 = pool.tile([1, TQ], F32, name="ln_ex2", tag=tagp + "ex2")
    nc.scalar.activation(ex2[:], ps_sq[:], ACTF.Copy, scale=1.0 / n)
    return mean, ex2


def _ln_apply(nc, tc, pool, pbc, ones_sb, mean, ex2, in_sb, out_sb,
              scales, biases, tagp):
    """out = (in - mean)/sqrt(var_unbiased + eps) * s + b, stats over C."""
    n = float(C)
    m2 = pool.tile([1, TQ], F32, name="ln_m2", tag=tagp + "m2")
    nc.vector.tensor_mul(m2[:], mean[:], mean[:])
    dv = pool.tile([1, TQ], F32, name="ln_d", tag=tagp + "d")
    nc.vector.tensor_sub(dv[:], ex2[:], m2[:])
    eps_sb = pool.tile([1, 1], F32, name="ln_eps", tag=tagp + "eps")
    nc.vector.memset(eps_sb[:], float(EPS))
    std = pool.tile([1, TQ], F32, name="ln_std", tag=tagp + "std")
    nc.scalar.activation(std[:], dv[:], ACTF.Sqrt,
                         scale=n / (n - 1.0), bias=eps_sb[:])
    istd = pool.tile([1, TQ], F32R, name="ln_istd", tag=tagp + "istd")
    with nc.allow_low_precision(reason="f32r matmul operand"):
        nc.vector.reciprocal(istd[:], std[:])
    # broadcast mean and istd across partitions via K=1 matmul
    pmb = pbc.tile([128, TQ], F32, name="ln_pmb", tag="bc")
    nc.tensor.matmul(pmb[:], ones_sb[0:1, :], mean[:], start=True, stop=True)
    mb = pool.tile([128, TQ], F32, name="ln_mb", tag=tagp + "mb")
    nc.scalar.activation(mb[:], pmb[:], ACTF.Copy)
    pib = pbc.tile([128, TQ], F32, name="ln_pib", tag="bc")
    nc.tensor.matmul(pib[:], ones_sb[0:1, :], istd[:], start=True, stop=True)
    ib = pool.tile([128, TQ], F32, name="ln_ib", tag=tagp + "ib")
    nc.scalar.activation(ib[:], pib[:], ACTF.Copy)
    for cc in range(CC):
        t1 = pool.tile([128, TQ], F32, name="ln_t1", tag=tagp + "t1")
        nc.vector.tensor_sub(t1[:], in_sb[cc][:], mb[:])
        t2 = pool.tile([128, TQ], F32, name="ln_t2", tag=tagp + "t2")
        nc.vector.tensor_mul(t2[:], t1[:], ib[:])
        nc.vector.tensor_scalar(out_sb[cc][:], t2[:], scales[cc], biases[cc],
                                mybir.AluOpType.mult, mybir.AluOpType.add)


def _emit(nc, tc, d, taps):
    # ---- persistent constants ------------------------------------------
    const = tc.alloc_tile_pool(name="const", bufs=1, side="left")
    ones_sb = const.tile([128, 128], F32R, name="ones_sb")
    prk_sb = const.tile([128, CC, 8], F32, name="prk_sb")
    b1p_sb = const.tile([128, ICN], F32, name="b1p_sb")
    bvb_sb = const.tile([128, C], F32, name="bvb_sb")

    def prm(cc, pi):
        return prk_sb[:, cc, pi].unsqueeze(-1)  # [128,1]

    xtq_pool = tc.alloc_tile_pool(name="xtq", bufs=1, side="left")
    xtq_sb = []
    for cc in range(CC):
        t = xtq_pool.tile([128, TQ], F32R, name=f"xtq{cc}")
        nc.sync.dma_start(out=t[:], in_=d["xtq"][cc * 128:(cc + 1) * 128, :])
        xtq_sb.append(t)

    nc.sync.dma_start(out=ones_sb[:], in_=d["ones"][:])
    nc.sync.dma_start(out=prk_sb[:], in_=d["prk"][:])
    nc.sync.dma_start(out=b1p_sb[:], in_=d["b1p"][:])
    nc.sync.dma_start(out=bvb_sb[:], in_=d["bvb"][:])

    qT_pool = tc.alloc_tile_pool(name="qTp", bufs=1, side="left")
    qz_sb = [qT_pool.tile([128, TQ], F32R, name=f"qz{h}") for h in range(H)]

    kv_pool = tc.alloc_tile_pool(name="kvp", bufs=1, side="left")
    kT_sb = [kv_pool.tile([128, T], F32R, name=f"kT{cc}") for cc in range(CC)]
    v_sb = [kv_pool.tile([128, H, DH + 1], BF16, name=f"v{tch}")
            for tch in range(KCN)]

    # ==================== phase A: QKV ==================================
    with tc.tile_pool(name="wstr", bufs=13, side="right") as wpool, \
         tc.tile_pool(name="xts", bufs=8, side="right") as xt_pool, \
         tc.tile_pool(name="pqkv", bufs=2, space="PSUM") as pqkv:

        wq_sb = []
        for cc in range(CC):
            t = wpool.tile([128, C], F32R, name="w_t", tag="w")
            nc.sync.dma_start(out=t[:], in_=d["wq"][cc * 128:(cc + 1) * 128, :])
            wq_sb.append(t)
        # q^T = Wq^T x_q^T + bq; per-head rows ro..ro+DH of qz, rest zero
        for mc in range(CC):
            ps = pqkv.tile([128, TQ], F32, name="ps_q", tag="pq")
            for kc in range(CC):
                nc.tensor.matmul(ps[:], wq_sb[kc][:, mc * 128:(mc + 1) * 128],
                                 xtq_sb[kc][:],
                                 start=(kc == 0), stop=(kc == CC - 1))
            for half in range(2):
                h = 2 * mc + half
                ro = half * DH
                nc.vector.tensor_scalar_add(
                    qz_sb[h][ro:ro + DH, :], ps[ro:ro + DH, :],
                    prm(mc, P_BQ)[ro:ro + DH, :])

        wk_sb = []
        for cc in range(CC):
            t = wpool.tile([128, C], F32R, name="w_t", tag="w")
            nc.sync.dma_start(out=t[:], in_=d["wk"][cc * 128:(cc + 1) * 128, :])
            wk_sb.append(t)
        wv_sb = []
        for cc in range(CC):
            t = wpool.tile([128, C], F32R, name="w_t", tag="w")
            nc.sync.dma_start(out=t[:], in_=d["wv"][cc * 128:(cc + 1) * 128, :])
            wv_sb.append(t)

        for tb in range(TBN):
            xt_blk = []
            for cc in range(CC):
                t = xt_pool.tile([128, 512], F32R, name="xt_t", tag="xt")
                nc.sync.dma_start(
                    out=t[:], in_=d["xt"][cc * 128:(cc + 1) * 128,
                                          tb * 512:(tb + 1) * 512])
                xt_blk.append(t)
            # k^T columns of this block
            for mc in range(CC):
                ps = pqkv.tile([128, 512], F32, name="ps_k", tag="pq")
                for kc in range(CC):
                    nc.tensor.matmul(ps[:],
                                     wk_sb[kc][:, mc * 128:(mc + 1) * 128],
                                     xt_blk[kc][:],
                                     start=(kc == 0), stop=(kc == CC - 1))
                nc.vector.tensor_scalar_add(
                    kT_sb[mc][:, tb * 512:(tb + 1) * 512], ps[:],
                    prm(mc, P_BK))
            # v rows (natural layout), 4 chunks of 128 tokens each
            for tci in range(4):
                tch = tb * 4 + tci
                ps1 = pqkv.tile([128, 512], F32, name="ps_v1", tag="pv1")
                ps2 = pqkv.tile([128, 256], F32, name="ps_v2", tag="pv2")
                for kc in range(CC):
                    xsl = xt_blk[kc][:, tci * 128:(tci + 1) * 128]
                    nc.tensor.matmul(ps1[:], xsl, wv_sb[kc][:, 0:512],
                                     start=(kc == 0), stop=(kc == CC - 1))
                    nc.tensor.matmul(ps2[:], xsl, wv_sb[kc][:, 512:C],
                                     start=(kc == 0), stop=(kc == CC - 1))
                vt = v_sb[tch]
                nc.vector.tensor_add(
                    vt[:, 0:8, 0:DH],
                    ps1[:].rearrange("p (h d) -> p h d", d=DH),
                    bvb_sb[:, 0:512].rearrange("p (h d) -> p h d", d=DH))
                nc.vector.tensor_add(
                    vt[:, 8:H, 0:DH],
                    ps2[:].rearrange("p (h d) -> p h d", d=DH),
                    bvb_sb[:, 512:C].rearrange("p (h d) -> p h d", d=DH))
                nc.vector.tensor_copy(vt[:, :, DH], ones_sb[:, 0:H])

    if taps:
        _tap(nc, tc, taps["qT"], [t[:] for t in qT_sb], TQ)
        _tap(nc, tc, taps["kT"], [t[:] for t in kT_sb], T)
        _tap(nc, tc, taps["v"],
             [v_sb[tch][:, :, 0:DH] for tch in range(KCN)], C)

    # ==================== phase B: attention ============================
    for h in range(H):
        ro = (h % 2) * DH
        nc.sync.dma_start(out=qz_sb[h][(DH - ro):(128 - ro), :],
                          in_=d["zer"][:])
    ctxT_pool = tc.alloc_tile_pool(name="ctxTp", bufs=1, side="right")
    ctxT_sb = [ctxT_pool.tile([128, TQ], F32R, name=f"ctxT{cc}")
               for cc in range(CC)]

    with tc.tile_pool(name="mskp", bufs=1, side="right") as mpool, \
         tc.tile_pool(name="attnp", bufs=6, side="right") as apool, \
         tc.tile_pool(name="pctx", bufs=1, space="PSUM") as pctx, \
         tc.tile_pool(name="psc", bufs=2, space="PSUM") as psc:
        msk_sb = []
        for kc2 in range(KCN // 2):
            mt = mpool.tile([128, 2 * TQ], BF16, name=f"msk{kc2}")
            nc.sync.dma_start(out=mt[:], in_=d["msk"][kc2, :, :])
            msk_sb.append(mt)
        dn_sb = mpool.tile([65, 4 * TQ], F32, name="dn_sb")
        rcp_all = mpool.tile([65, 4 * TQ], F32R, name="rcp_all")
        nc.vector.memset(dn_sb[:], 1.0)

        pending = []

        def _norm_ops_for_group(g):
            ops = []
            for q in range(4):
                def _recip(g=g, q=q):
                    c0 = g * TQ + q * (TQ // 4)
                    with nc.allow_low_precision(reason="f32r operand"):
                        nc.vector.reciprocal(rcp_all[:, c0:c0 + TQ // 4],
                                             dn_sb[:, c0:c0 + TQ // 4])
                ops.append(_recip)
            for hh in range(g * 3, g * 3 + 3):
                def _one(hh=hh, g=g):
                    cc2, ro2 = hh // 2, (hh % 2) * DH
                    bp2 = (hh % 3) * 32
                    pb = pctx.tile([DH, TQ], F32, name="pb", tag="pb",
                                   bufs=1)
                    nc.tensor.matmul(
                        pb[:DH, :], ones_sb[bp2:bp2 + 1, 0:DH],
                        rcp_all[bp2:bp2 + 1, g * TQ:(g + 1) * TQ],
                        start=True, stop=True)
                    bc = apool.tile([128, TQ], F32, name="bc", tag="bc")
                    nc.scalar.activation(bc[ro2:ro2 + DH, :], pb[:DH, :],
                                         ACTF.Copy)
                    nc.vector.tensor_mul(ctxT_sb[cc2][ro2:ro2 + DH, :],
                                         ctxT_sb[cc2][ro2:ro2 + DH, :],
                                         bc[ro2:ro2 + DH, :])
                ops.append(_one)
            return ops

        for h in range(H):
            cc, ro = h // 2, (h % 2) * DH
            ctx_ps = pctx.tile([DH + 1, TQ], F32, name="ctx_ps", tag="ctx",
                               bufs=3)
            for kc2 in range(KCN // 2):
                # causal: chunk kc only reaches queries qq >= 32*kc (uniform
                # across cores). Skip the fully-masked left part; pack the
                # two halves contiguously so exp reads one gap-free region.
                s0 = min(64 * kc2, 256)          # half-0 skip (f32r N>=256)
                sl1 = min(64 * kc2 + 32, 256)    # half-1 skip
                ps = psc.tile([128, 2 * TQ], F32, name="ps_s", tag="s")
                nc.tensor.matmul(
                    ps[:, s0:TQ],
                    kT_sb[cc][:, (2 * kc2) * 128:(2 * kc2 + 1) * 128],
                    qz_sb[h][:, s0:], start=True, stop=True)
                nc.tensor.matmul(
                    ps[:, TQ:2 * TQ - sl1],
                    kT_sb[cc][:, (2 * kc2 + 1) * 128:(2 * kc2 + 2) * 128],
                    qz_sb[h][:, sl1:], start=True, stop=True)
                et = apool.tile([128, 2 * TQ], BF16, name="et", tag="e")
                nc.scalar.activation(et[:, s0:2 * TQ - sl1],
                                     ps[:, s0:2 * TQ - sl1], ACTF.Exp,
                                     scale=float(SCALE))
                nc.vector.tensor_mul(et[:, s0:2 * TQ - sl1],
                                     et[:, s0:2 * TQ - sl1],
                                     msk_sb[kc2][:, s0:2 * TQ - sl1])
                for half in range(2):
                    kc = kc2 * 2 + half
                    qoff = 32 * kc
                    if half == 0:
                        rsl = slice(qoff, TQ)
                    else:
                        rsl = slice(TQ + qoff - sl1, 2 * TQ - sl1)
                    nc.tensor.matmul(ctx_ps[:, qoff:], v_sb[kc][:, h, :],
                                     et[:, rsl],
                                     start=(kc == 0), stop=(kc == KCN - 1))
                if pending:
                    pending.pop(0)()
            # evict unnormalized ctx + denominator row; normalize later
            nc.scalar.activation(ctxT_sb[cc][ro:ro + DH, :],
                                 ctx_ps[0:DH, :], ACTF.Copy)
            bp, g = (h % 3) * 32, h // 3
            nc.vector.tensor_copy(dn_sb[bp:bp + 1, g * TQ:(g + 1) * TQ],
                                  ctx_ps[DH:DH + 1, :])
            if h % 3 == 2:
                pending.extend(_norm_ops_for_group(h // 3))
        for op in pending:
            op()

    kv_pool.release()
    qT_pool.release()

    if taps:
        _tap(nc, tc, taps["ctxT"], [t[:] for t in ctxT_sb], TQ)

    # ==================== phase C: Wo + residual + LN1 ==================
    w1pool = tc.alloc_tile_pool(name="w1pool", bufs=2 * CC, side="right")
    hT_holder = {}

    with tc.tile_pool(name="cpool", bufs=2, side="right") as cpool, \
         tc.tile_pool(name="wopool", bufs=7, side="right") as wopool, \
         tc.tile_pool(name="r1pool", bufs=1, side="right") as r1pool:
        wo_sb = []
        for cc in range(CC):
            t = wopool.tile([128, C], F32R, name="wo_t", tag="wo")
            nc.sync.dma_start(out=t[:], in_=d["wo"][cc * 128:(cc + 1) * 128, :])
            wo_sb.append(t)
        r1_sb = [r1pool.tile([128, TQ], F32R, name=f"r1{cc}")
                 for cc in range(CC)]
        with tc.tile_pool(name="pao", bufs=2, space="PSUM") as pao, \
             tc.tile_pool(name="pst", bufs=2, space="PSUM") as pst:
            ps_sum = pst.tile([1, TQ], F32, name="ps_sum", tag="st")
            ps_sq = pst.tile([1, TQ], F32, name="ps_sq", tag="st")
            for mc in range(CC):
                ps = pao.tile([128, TQ], F32, name="ps_ao", tag="ao")
                for kc in range(CC):
                    nc.tensor.matmul(ps[:],
                                     wo_sb[kc][:, mc * 128:(mc + 1) * 128],
                                     ctxT_sb[kc][:],
                                     start=(kc == 0), stop=(kc == CC - 1))
                nc.vector.scalar_tensor_tensor(
                    r1_sb[mc][:], ps[:], prm(mc, P_BO), xtq_sb[mc][:],
                    mybir.AluOpType.add, mybir.AluOpType.add)
                nc.tensor.matmul(ps_sum[:], ones_sb[:, 0:1], r1_sb[mc][:],
                                 start=(mc == 0), stop=(mc == CC - 1))
                sq = cpool.tile([128, TQ], F32R, name="sq", tag="sq")
                nc.scalar.activation(sq[:], r1_sb[mc][:], ACTF.Square)
                nc.tensor.matmul(ps_sq[:], ones_sb[:, 0:1], sq[:],
                                 start=(mc == 0), stop=(mc == CC - 1))
            mean1, ex21 = _ln_stats(nc, cpool, ps_sum, ps_sq, "l1")
        xtq_pool.release()
        hT_pool = tc.alloc_tile_pool(name="hTp", bufs=1, side="left")
        hT_sb = [hT_pool.tile([128, TQ], F32R, name=f"hT{cc}")
                 for cc in range(CC)]
        hT_holder["pool"] = hT_pool
        hT_holder["tiles"] = hT_sb
        with tc.tile_pool(name="pbc2", bufs=2, space="PSUM") as pbc2:
            _ln_apply(nc, tc, cpool, pbc2, ones_sb, mean1, ex21, r1_sb, hT_sb,
                      [prm(cc, P_L1S) for cc in range(CC)],
                      [prm(cc, P_L1B) for cc in range(CC)], "l1")

    if taps:
        _tap(nc, tc, taps["hT"], [t[:] for t in hT_sb], TQ)

    # ==================== phase D: MLP + residual + LN2 =================
    with tc.tile_pool(name="dpool", bufs=3, side="right") as dpool, \
         tc.tile_pool(name="w2pool", bufs=3, side="right") as w2pool, \
         tc.tile_pool(name="r2pool", bufs=1, side="right") as r2pool:

        r2_sb = [r2pool.tile([128, TQ], F32R, name=f"r2{cc}")
                 for cc in range(CC)]
        with tc.tile_pool(name="pfc2", bufs=1, space="PSUM") as pfc2:
            ps_m = [pfc2.tile([128, TQ], F32, name=f"ps_m{mc}", tag=f"m{mc}")
                    for mc in range(CC)]
            with tc.tile_pool(name="pfc1", bufs=2, space="PSUM") as pfc1:
                w1blk = {}
                for kc2 in range(ICN):
                    jb = kc2 // CC
                    if kc2 % CC == 0:
                        w1blk[jb] = []
                        for kc in range(CC):
                            t = w1pool.tile([128, C], F32R, name="w1_t",
                                            tag="w1")
                            nc.sync.dma_start(
                                out=t[:],
                                in_=d["w1"][kc * 128:(kc + 1) * 128,
                                            jb * C:(jb + 1) * C])
                            w1blk[jb].append(t)
                    w2t = w2pool.tile([128, C], F32R, name="w2_t", tag="w2")
                    nc.sync.dma_start(
                        out=w2t[:], in_=d["w2"][kc2 * 128:(kc2 + 1) * 128, :])
                    ps1 = pfc1.tile([128, TQ], F32, name="ps1", tag="f1")
                    co = (kc2 % CC) * 128
                    for kc in range(CC):
                        nc.tensor.matmul(
                            ps1[:], w1blk[jb][kc][:, co:co + 128],
                            hT_sb[kc][:],
                            start=(kc == 0), stop=(kc == CC - 1))
                    g = dpool.tile([128, TQ], F32R, name="g", tag="g")
                    nc.scalar.activation(g[:], ps1[:], ACTF.Gelu_apprx_tanh,
                                         bias=b1p_sb[:, kc2].unsqueeze(-1))
                    for mc in range(CC):
                        nc.tensor.matmul(ps_m[mc][:],
                                         w2t[:, mc * 128:(mc + 1) * 128],
                                         g[:], start=(kc2 == 0),
                                         stop=(kc2 == ICN - 1))
            with tc.tile_pool(name="pst2", bufs=2, space="PSUM") as pst2:
                ps_sum2 = pst2.tile([1, TQ], F32, name="ps_sum2", tag="st")
                ps_sq2 = pst2.tile([1, TQ], F32, name="ps_sq2", tag="st")
                for mc in range(CC):
                    nc.vector.scalar_tensor_tensor(
                        r2_sb[mc][:], ps_m[mc][:], prm(mc, P_B2),
                        hT_sb[mc][:], mybir.AluOpType.add,
                        mybir.AluOpType.add)
                    nc.tensor.matmul(ps_sum2[:], ones_sb[:, 0:1], r2_sb[mc][:],
                                     start=(mc == 0), stop=(mc == CC - 1))
                    sq = dpool.tile([128, TQ], F32R, name="sq2", tag="sq")
                    nc.scalar.activation(sq[:], r2_sb[mc][:], ACTF.Square)
                    nc.tensor.matmul(ps_sq2[:], ones_sb[:, 0:1], sq[:],
                                     start=(mc == 0), stop=(mc == CC - 1))
                mean2, ex22 = _ln_stats(nc, dpool, ps_sum2, ps_sq2, "l2")
        hT_pool.release()
        with tc.tile_pool(name="pbc3", bufs=2, space="PSUM") as pbc3:
            outT_sb = [dpool.tile([128, TQ], F32, name=f"o{cc}", tag=f"o{cc}",
                                  bufs=1) for cc in range(CC)]
            _ln_apply(nc, tc, dpool, pbc3, ones_sb, mean2, ex22, r2_sb,
                      outT_sb,
                      [prm(cc, P_L2S) for cc in range(CC)],
                      [prm(cc, P_L2B) for cc in range(CC)], "l2")
            for cc in range(CC):
                nc.sync.dma_start(out=d["outT"][cc * 128:(cc + 1) * 128, :],
                                  in_=outT_sb[cc][:])

    w1pool.release()
    ctxT_pool.release()
    const.release()


_NC = None


def _get_nc():
    global _NC
    if _NC is None:
        _NC = _build_nc()
    return _NC


def _prep_inmaps(x, Wq, bq, Wk, bk, Wv, bv, Wo, bo, ln1_s, ln1_b,
                 W1, b1, W2, b2, ln2_s, ln2_b):
    f32 = np.float32
    xT = [np.ascontiguousarray(np.asarray(x)[b].T, dtype=f32)
          for b in range(B)]
    wq = np.ascontiguousarray(Wq, dtype=f32)
    wk = np.ascontiguousarray(Wk, dtype=f32)
    wv = np.ascontiguousarray(Wv, dtype=f32)
    wo = np.ascontiguousarray(Wo, dtype=f32)
    w1 = np.ascontiguousarray(W1, dtype=f32)
    w2 = np.ascontiguousarray(W2, dtype=f32)
    prk = np.zeros((128, CC, 8), f32)
    for pi, arr in ((P_BQ, bq), (P_BK, bk), (P_BO, bo), (P_B2, b2),
                    (P_L1S, ln1_s), (P_L1B, ln1_b), (P_L2S, ln2_s),
                    (P_L2B, ln2_b)):
        prk[:, :, pi] = np.asarray(arr, f32).reshape(CC, 128).T
    b1p = np.ascontiguousarray(np.asarray(b1, f32).reshape(ICN, 128).T)
    bvb = np.broadcast_to(np.asarray(bv, f32)[None, :], (128, C)).copy()
    ones = np.ones((128, 128), f32)
    kk = np.arange(128)[:, None]
    qq = np.arange(TQ)[None, :]
    in_maps = []
    for c in range(8):
        b, p = c // 4, c % 4
        msk = np.zeros((KCN // 2, 128, 2 * TQ), ml_dtypes.bfloat16)
        for kc2 in range(KCN // 2):
            sl1 = min(64 * kc2 + 32, 256)
            m0 = ((128 * (2 * kc2) + kk) <= (p + 4 * qq))
            m1 = ((128 * (2 * kc2 + 1) + kk) <= (p + 4 * qq))
            msk[kc2, :, 0:TQ] = m0.astype(ml_dtypes.bfloat16)
            msk[kc2, :, TQ:2 * TQ - sl1] = m1[:, sl1:].astype(
                ml_dtypes.bfloat16)
        in_maps.append({
            "xt": xT[b], "xtq": np.ascontiguousarray(xT[b][:, p::4]),
            "wq": wq, "wk": wk, "wv": wv, "wo": wo, "w1": w1, "w2": w2,
            "prk": prk, "b1p": b1p, "bvb": bvb, "msk": msk, "ones": ones,
            "zer": np.zeros((DH, TQ), f32),
        })
    return in_maps


def _run(in_maps, trace=False, **kw):
    nc = _get_nc()
    return run_bass_kernel_spmd(nc, in_maps, list(range(8)), trace=trace, **kw)


def kernel(**inputs):
    in_maps = _prep_inmaps(**inputs)
    res = _run(in_maps)
    out = np.empty((B, T, C), np.float32)
    for c in range(8):
        b, p = c // 4, c % 4
        out[b, p::4, :] = res.results[c]["outT"].T
    return out


# revision 33
# speedup vs baseline: 1.0561x; 1.0561x over previous
"""Trainium2 Bass kernel for one transformer block (B=2, T=2048, C=768, H=12,
inner=3072, fp32, causal, post-norm residual).

Sharding: 8 cores, token-interleaved. Core c handles batch c//4, tokens
p::4 (p = c%4) of that batch — every core runs the IDENTICAL program
(SPMD), causality is entirely data-driven via per-core mask tensors.

Layout: all activations transposed [C, tokens] so matmul contractions land
on partitions. K/V are (redundantly) computed for the full 2048 tokens on
every core; attention runs as scores^T -> exp -> [V|ones] matmul which
accumulates ctx^T and the softmax denominator in one pass (no max
subtraction needed: scores are ~N(0,1), |s|max << fp32 exp range).
Partition-dim reductions (softmax sums, layernorm stats) use ones-vector
matmuls on the PE; partition broadcasts use K=1 outer-product matmuls.

Matmuls run in float32r (tf32-like, full PE rate at N>=256); everything
else fp32.
"""

import sys

if "/opt/trn_rl_repo" not in sys.path:
    sys.path.insert(0, "/opt/trn_rl_repo")

import numpy as np
import ml_dtypes

import concourse.bacc as bacc
import concourse.mybir as mybir
import concourse.tile as tile
from concourse.bass_utils import run_bass_kernel_spmd

F32 = mybir.dt.float32
F32R = mybir.dt.float32r
BF16 = mybir.dt.bfloat16
ACTF = mybir.ActivationFunctionType

B, T, C = 2, 2048, 768
H, DH = 12, 64
IN = 3072
CC = C // 128          # 6 channel chunks
TBN = T // 512         # 4 token blocks of full seq
TQ = 512               # tokens per core
KCN = T // 128         # 16 k-chunks
ICN = IN // 128        # 24 inner chunks
HG = 2                 # head groups
HPG = H // HG          # 6 heads per group
EPS = 1e-4
SCALE = 1.0 / np.sqrt(DH)

# param pack order in "prk" [128, CC, 8]
P_BQ, P_BK, P_BO, P_B2, P_L1S, P_L1B, P_L2S, P_L2B = range(8)

DEBUG_TAPS = False


def _build_nc():
    nc = bacc.Bacc("TRN2", target_bir_lowering=False, debug=False,
                   enable_asserts=False, num_devices=8)
    d = {}
    d["xt"] = nc.dram_tensor("xt", [C, T], F32R, kind="ExternalInput").ap()
    d["xtq"] = nc.dram_tensor("xtq", [C, TQ], F32R, kind="ExternalInput").ap()
    for w, sh in (("wq", [C, C]), ("wk", [C, C]), ("wv", [C, C]),
                  ("wo", [C, C]), ("w1", [C, IN]), ("w2", [IN, C])):
        d[w] = nc.dram_tensor(w, sh, F32R, kind="ExternalInput").ap()
    d["prk"] = nc.dram_tensor("prk", [128, CC, 8], F32, kind="ExternalInput").ap()
    d["b1p"] = nc.dram_tensor("b1p", [128, ICN], F32, kind="ExternalInput").ap()
    d["bvb"] = nc.dram_tensor("bvb", [128, C], F32, kind="ExternalInput").ap()
    d["msk"] = nc.dram_tensor("msk", [4, 128, 2 * TQ], BF16, kind="ExternalInput").ap()
    d["mskq"] = nc.dram_tensor("mskq", [2, 128, 2 * TQ], BF16, kind="ExternalInput").ap()
    d["ones"] = nc.dram_tensor("ones", [128, 128], F32R, kind="ExternalInput").ap()
    d["zer"] = nc.dram_tensor("zer", [DH, TQ], F32R, kind="ExternalInput").ap()
    d["outT"] = nc.dram_tensor("outT", [C, TQ], F32, kind="ExternalOutput").ap()
    taps = {}
    if DEBUG_TAPS:
        taps["qT"] = nc.dram_tensor("qT", [C, TQ], F32, kind="ExternalOutput").ap()
        taps["kT"] = nc.dram_tensor("kT", [C, T], F32, kind="ExternalOutput").ap()
        taps["v"] = nc.dram_tensor("v", [T, C], F32, kind="ExternalOutput").ap()
        taps["ctxT"] = nc.dram_tensor("ctxT", [C, TQ], F32, kind="ExternalOutput").ap()
        taps["hT"] = nc.dram_tensor("hT", [C, TQ], F32, kind="ExternalOutput").ap()

    with tile.TileContext(nc) as tc:
        _emit(nc, tc, d, taps)
    nc.finalize()
    return nc


def _tap(nc, tc, dst, tiles, width):
    with tc.tile_pool(name="tapp", bufs=2, side="right") as tp:
        for i, src in enumerate(tiles):
            t = tp.tile([128, width], F32, name="tapt", tag="t")
            nc.vector.tensor_copy(t[:], src)
            nc.sync.dma_start(out=dst[i * 128:(i + 1) * 128, :], in_=t[:])


def _ln_stats(nc, pool, ps_sum, ps_sq, tagp):
    """ACT-copy psum stats rows to SBUF (inside the psum pool's scope)."""
    n = float(C)
    mean = pool.tile([1, TQ], F32R, name="ln_mean", tag=tagp + "mean")
    nc.scalar.activation(mean[:], ps_sum[:], ACTF.Copy, scale=1.0 / n)
    ex2 = pool.tile([1, TQ], F32, name="ln_ex2", tag=tagp + "ex2")
    nc.scalar.activation(ex2[:], ps_sq[:], ACTF.Copy, scale=1.0 / n)
    return mean, ex2


def _ln_apply(nc, tc, pool, pbc, ones_sb, mean, ex2, in_sb, out_sb,
              scales, biases, tagp):
    """out = (in - mean)/sqrt(var_unbiased + eps) * s + b, stats over C."""
    n = float(C)
    m2 = pool.tile([1, TQ], F32, name="ln_m2", tag=tagp + "m2")
    nc.vector.tensor_mul(m2[:], mean[:], mean[:])
    dv = pool.tile([1, TQ], F32, name="ln_d", tag=tagp + "d")
    nc.vector.tensor_sub(dv[:], ex2[:], m2[:])
    eps_sb = pool.tile([1, 1], F32, name="ln_eps", tag=tagp + "eps")
    nc.vector.memset(eps_sb[:], float(EPS))
    std = pool.tile([1, TQ], F32, name="ln_std", tag=tagp + "std")
    nc.scalar.activation(std[:], dv[:], ACTF.Sqrt,
                         scale=n / (n - 1.0), bias=eps_sb[:])
    istd = pool.tile([1, TQ], F32R, name="ln_istd", tag=tagp + "istd")
    with nc.allow_low_precision(reason="f32r matmul operand"):
        nc.vector.reciprocal(istd[:], std[:])
    # broadcast mean and istd across partitions via K=1 matmul
    pmb = pbc.tile([128, TQ], F32, name="ln_pmb", tag="bc")
    nc.tensor.matmul(pmb[:], ones_sb[0:1, :], mean[:], start=True, stop=True)
    mb = pool.tile([128, TQ], F32, name="ln_mb", tag=tagp + "mb")
    nc.scalar.activation(mb[:], pmb[:], ACTF.Copy)
    pib = pbc.tile([128, TQ], F32, name="ln_pib", tag="bc")
    nc.tensor.matmul(pib[:], ones_sb[0:1, :], istd[:], start=True, stop=True)
    ib = pool.tile([128, TQ], F32, name="ln_ib", tag=tagp + "ib")
    nc.scalar.activation(ib[:], pib[:], ACTF.Copy)
    for cc in range(CC):
        t1 = pool.tile([128, TQ], F32, name="ln_t1", tag=tagp + "t1")
        nc.vector.tensor_sub(t1[:], in_sb[cc][:], mb[:])
        t2 = pool.tile([128, TQ], F32, name="ln_t2", tag=tagp + "t2")
        nc.vector.tensor_mul(t2[:], t1[:], ib[:])
        nc.vector.tensor_scalar(out_sb[cc][:], t2[:], scales[cc], biases[cc],
                                mybir.AluOpType.mult, mybir.AluOpType.add)


def _emit(nc, tc, d, taps):
    # ---- persistent constants ------------------------------------------
    const = tc.alloc_tile_pool(name="const", bufs=1, side="left")
    ones_sb = const.tile([128, 128], F32R, name="ones_sb")
    prk_sb = const.tile([128, CC, 8], F32, name="prk_sb")
    b1p_sb = const.tile([128, ICN], F32, name="b1p_sb")
    bvb_sb = const.tile([128, C], F32, name="bvb_sb")

    def prm(cc, pi):
        return prk_sb[:, cc, pi].unsqueeze(-1)  # [128,1]

    xtq_pool = tc.alloc_tile_pool(name="xtq", bufs=1, side="left")
    xtq_sb = []
    for cc in range(CC):
        t = xtq_pool.tile([128, TQ], F32R, name=f"xtq{cc}")
        nc.sync.dma_start(out=t[:], in_=d["xtq"][cc * 128:(cc + 1) * 128, :])
        xtq_sb.append(t)

    nc.sync.dma_start(out=ones_sb[:], in_=d["ones"][:])
    nc.sync.dma_start(out=prk_sb[:], in_=d["prk"][:])
    nc.sync.dma_start(out=b1p_sb[:], in_=d["b1p"][:])
    nc.sync.dma_start(out=bvb_sb[:], in_=d["bvb"][:])

    qT_pool = tc.alloc_tile_pool(name="qTp", bufs=1, side="left")
    qz_sb = [qT_pool.tile([128, TQ], F32R, name=f"qz{h}") for h in range(H)]

    kv_pool = tc.alloc_tile_pool(name="kvp", bufs=1, side="left")
    kT_sb = [kv_pool.tile([128, T], F32R, name=f"kT{cc}") for cc in range(CC)]
    v_sb = [kv_pool.tile([128, H, DH + 1], BF16, name=f"v{tch}")
            for tch in range(KCN)]

    # ==================== phase A: QKV ==================================
    with tc.tile_pool(name="wstr", bufs=13, side="right") as wpool, \
         tc.tile_pool(name="xts", bufs=8, side="right") as xt_pool, \
         tc.tile_pool(name="pqkv", bufs=2, space="PSUM") as pqkv:

        wq_sb = []
        for cc in range(CC):
            t = wpool.tile([128, C], F32R, name="w_t", tag="w")
            nc.sync.dma_start(out=t[:], in_=d["wq"][cc * 128:(cc + 1) * 128, :])
            wq_sb.append(t)
        # q^T = Wq^T x_q^T + bq; per-head rows ro..ro+DH of qz, rest zero
        for mc in range(CC):
            ps = pqkv.tile([128, TQ], F32, name="ps_q", tag="pq")
            for kc in range(CC):
                nc.tensor.matmul(ps[:], wq_sb[kc][:, mc * 128:(mc + 1) * 128],
                                 xtq_sb[kc][:],
                                 start=(kc == 0), stop=(kc == CC - 1))
            for half in range(2):
                h = 2 * mc + half
                ro = half * DH
                nc.vector.tensor_scalar_add(
                    qz_sb[h][ro:ro + DH, :], ps[ro:ro + DH, :],
                    prm(mc, P_BQ)[ro:ro + DH, :])

        wk_sb = []
        for cc in range(CC):
            t = wpool.tile([128, C], F32R, name="w_t", tag="w")
            nc.sync.dma_start(out=t[:], in_=d["wk"][cc * 128:(cc + 1) * 128, :])
            wk_sb.append(t)
        wv_sb = []
        for cc in range(CC):
            t = wpool.tile([128, C], F32R, name="w_t", tag="w")
            nc.sync.dma_start(out=t[:], in_=d["wv"][cc * 128:(cc + 1) * 128, :])
            wv_sb.append(t)

        for tb in range(TBN):
            xt_blk = []
            for cc in range(CC):
                t = xt_pool.tile([128, 512], F32R, name="xt_t", tag="xt")
                nc.sync.dma_start(
                    out=t[:], in_=d["xt"][cc * 128:(cc + 1) * 128,
                                          tb * 512:(tb + 1) * 512])
                xt_blk.append(t)
            # k^T columns of this block
            for mc in range(CC):
                ps = pqkv.tile([128, 512], F32, name="ps_k", tag="pq")
                for kc in range(CC):
                    nc.tensor.matmul(ps[:],
                                     wk_sb[kc][:, mc * 128:(mc + 1) * 128],
                                     xt_blk[kc][:],
                                     start=(kc == 0), stop=(kc == CC - 1))
                nc.vector.tensor_scalar_add(
                    kT_sb[mc][:, tb * 512:(tb + 1) * 512], ps[:],
                    prm(mc, P_BK))
            # v rows (natural layout), 4 chunks of 128 tokens each
            for tci in range(4):
                tch = tb * 4 + tci
                ps1 = pqkv.tile([128, 512], F32, name="ps_v1", tag="pv1")
                ps2 = pqkv.tile([128, 256], F32, name="ps_v2", tag="pv2")
                for kc in range(CC):
                    xsl = xt_blk[kc][:, tci * 128:(tci + 1) * 128]
                    nc.tensor.matmul(ps1[:], xsl, wv_sb[kc][:, 0:512],
                                     start=(kc == 0), stop=(kc == CC - 1))
                    nc.tensor.matmul(ps2[:], xsl, wv_sb[kc][:, 512:C],
                                     start=(kc == 0), stop=(kc == CC - 1))
                vt = v_sb[tch]
                nc.vector.tensor_add(
                    vt[:, 0:8, 0:DH],
                    ps1[:].rearrange("p (h d) -> p h d", d=DH),
                    bvb_sb[:, 0:512].rearrange("p (h d) -> p h d", d=DH))
                nc.vector.tensor_add(
                    vt[:, 8:H, 0:DH],
                    ps2[:].rearrange("p (h d) -> p h d", d=DH),
                    bvb_sb[:, 512:C].rearrange("p (h d) -> p h d", d=DH))
                nc.vector.tensor_copy(vt[:, :, DH], ones_sb[:, 0:H])

    if taps:
        _tap(nc, tc, taps["qT"], [t[:] for t in qT_sb], TQ)
        _tap(nc, tc, taps["kT"], [t[:] for t in kT_sb], T)
        _tap(nc, tc, taps["v"],
             [v_sb[tch][:, :, 0:DH] for tch in range(KCN)], C)

    # ==================== phase B: attention ============================
    for h in range(H):
        ro = (h % 2) * DH
        nc.sync.dma_start(out=qz_sb[h][(DH - ro):(128 - ro), :],
                          in_=d["zer"][:])
    ctxT_pool = tc.alloc_tile_pool(name="ctxTp", bufs=1, side="right")
    ctxT_sb = [ctxT_pool.tile([128, TQ], F32R, name=f"ctxT{cc}")
               for cc in range(CC)]

    with tc.tile_pool(name="mskp", bufs=1, side="right") as mpool, \
         tc.tile_pool(name="attnp", bufs=6, side="right") as apool, \
         tc.tile_pool(name="pctx", bufs=2, space="PSUM") as pctx, \
         tc.tile_pool(name="psc", bufs=3, space="PSUM") as psc:
        msk_sb = []
        for kc2 in range(4):
            mt = mpool.tile([128, 2 * TQ], BF16, name=f"msk{kc2}")
            nc.sync.dma_start(out=mt[:], in_=d["msk"][kc2, :, :])
            msk_sb.append(mt)
        mskq_sb = []
        for j in range(2):
            mt = mpool.tile([128, 2 * TQ], BF16, name=f"mskq{j}")
            nc.sync.dma_start(out=mt[:], in_=d["mskq"][j, :, :])
            mskq_sb.append(mt)
        dn_sb = mpool.tile([65, 4 * TQ], F32, name="dn_sb")
        rcp_all = mpool.tile([65, 4 * TQ], F32R, name="rcp_all")
        nc.vector.memset(dn_sb[:], 1.0)

        pending = []

        def _norm_ops_for_group(g):
            ops = []
            for q in range(4):
                def _recip(g=g, q=q):
                    c0 = g * TQ + q * (TQ // 4)
                    with nc.allow_low_precision(reason="f32r operand"):
                        nc.vector.reciprocal(rcp_all[:, c0:c0 + TQ // 4],
                                             dn_sb[:, c0:c0 + TQ // 4])
                ops.append(_recip)
            for hh in range(g * 3, g * 3 + 3):
                def _one(hh=hh, g=g):
                    cc2, ro2 = hh // 2, (hh % 2) * DH
                    bp2 = (hh % 3) * 32
                    pb = pctx.tile([DH, TQ], F32, name="pb", tag="ctx")
                    nc.tensor.matmul(
                        pb[:DH, :], ones_sb[bp2:bp2 + 1, 0:DH],
                        rcp_all[bp2:bp2 + 1, g * TQ:(g + 1) * TQ],
                        start=True, stop=True)
                    bc = apool.tile([128, TQ], F32, name="bc", tag="bc")
                    nc.scalar.activation(bc[ro2:ro2 + DH, :], pb[:DH, :],
                                         ACTF.Copy)
                    nc.vector.tensor_mul(ctxT_sb[cc2][ro2:ro2 + DH, :],
                                         ctxT_sb[cc2][ro2:ro2 + DH, :],
                                         bc[ro2:ro2 + DH, :])
                ops.append(_one)
            return ops

        for h in range(H):
            cc, ro = h // 2, (h % 2) * DH
            ctx_ps = pctx.tile([DH + 1, TQ], F32, name="ctx_ps", tag="ctx")
            for kc2 in range(4):
                # causal: chunk kc only reaches queries qq >= 32*kc (uniform
                # across cores). Skip the fully-masked left part; pack the
                # two halves contiguously so exp reads one gap-free region.
                s0 = 64 * kc2
                sl1 = 64 * kc2 + 32
                ps = psc.tile([128, 2 * TQ], F32, name="ps_s", tag="s")
                nc.tensor.matmul(
                    ps[:, s0:TQ],
                    kT_sb[cc][:, (2 * kc2) * 128:(2 * kc2 + 1) * 128],
                    qz_sb[h][:, s0:], start=True, stop=True)
                nc.tensor.matmul(
                    ps[:, TQ:2 * TQ - sl1],
                    kT_sb[cc][:, (2 * kc2 + 1) * 128:(2 * kc2 + 2) * 128],
                    qz_sb[h][:, sl1:], start=True, stop=True)
                et = apool.tile([128, 2 * TQ], BF16, name="et", tag="e")
                nc.scalar.activation(et[:, s0:2 * TQ - sl1],
                                     ps[:, s0:2 * TQ - sl1], ACTF.Exp,
                                     scale=float(SCALE))
                nc.vector.tensor_mul(et[:, s0:2 * TQ - sl1],
                                     et[:, s0:2 * TQ - sl1],
                                     msk_sb[kc2][:, s0:2 * TQ - sl1])
                for half in range(2):
                    kc = kc2 * 2 + half
                    qoff = 32 * kc
                    if half == 0:
                        rsl = slice(qoff, TQ)
                    else:
                        rsl = slice(TQ + qoff - sl1, 2 * TQ - sl1)
                    nc.tensor.matmul(ctx_ps[:, qoff:], v_sb[kc][:, h, :],
                                     et[:, rsl],
                                     start=(kc == 0), stop=False)
                if pending:
                    pending.pop(0)()
            for qd in range(2, 4):
                # chunks kc >= 8: each contributes 256 score columns
                # (f32r floor); pack four chunks into one [128,1024] tile
                # so a single exp covers them with no per-call overhead x4
                ps = psc.tile([128, 2 * TQ], F32, name="ps_s", tag="s")
                for i in range(4):
                    kc = qd * 4 + i
                    nc.tensor.matmul(
                        ps[:, i * 256:(i + 1) * 256],
                        kT_sb[cc][:, kc * 128:(kc + 1) * 128],
                        qz_sb[h][:, 256:], start=True, stop=True)
                et = apool.tile([128, 2 * TQ], BF16, name="et", tag="e")
                nc.scalar.activation(et[:], ps[:], ACTF.Exp,
                                     scale=float(SCALE))
                nc.vector.tensor_mul(et[:], et[:], mskq_sb[qd - 2][:])
                for i in range(4):
                    kc = qd * 4 + i
                    qoff = 32 * kc
                    rsl = slice(i * 256 + qoff - 256, (i + 1) * 256)
                    nc.tensor.matmul(ctx_ps[:, qoff:], v_sb[kc][:, h, :],
                                     et[:, rsl],
                                     start=False, stop=(kc == KCN - 1))
                if pending:
                    pending.pop(0)()
            # evict unnormalized ctx + denominator row; normalize later
            nc.scalar.activation(ctxT_sb[cc][ro:ro + DH, :],
                                 ctx_ps[0:DH, :], ACTF.Copy)
            bp, g = (h % 3) * 32, h // 3
            nc.vector.tensor_copy(dn_sb[bp:bp + 1, g * TQ:(g + 1) * TQ],
                                  ctx_ps[DH:DH + 1, :])
            if h % 3 == 2:
                pending.extend(_norm_ops_for_group(h // 3))
        for op in pending:
            op()

    kv_pool.release()
    qT_pool.release()

    if taps:
        _tap(nc, tc, taps["ctxT"], [t[:] for t in ctxT_sb], TQ)

    # ==================== phase C: Wo + residual + LN1 ==================
    w1pool = tc.alloc_tile_pool(name="w1pool", bufs=2 * CC, side="right")
    hT_holder = {}

    with tc.tile_pool(name="cpool", bufs=2, side="right") as cpool, \
         tc.tile_pool(name="wopool", bufs=7, side="right") as wopool, \
         tc.tile_pool(name="r1pool", bufs=1, side="right") as r1pool:
        wo_sb = []
        for cc in range(CC):
            t = wopool.tile([128, C], F32R, name="wo_t", tag="wo")
            nc.sync.dma_start(out=t[:], in_=d["wo"][cc * 128:(cc + 1) * 128, :])
            wo_sb.append(t)
        r1_sb = [r1pool.tile([128, TQ], F32R, name=f"r1{cc}")
                 for cc in range(CC)]
        with tc.tile_pool(name="pao", bufs=2, space="PSUM") as pao, \
             tc.tile_pool(name="pst", bufs=2, space="PSUM") as pst:
            ps_sum = pst.tile([1, TQ], F32, name="ps_sum", tag="st")
            ps_sq = pst.tile([1, TQ], F32, name="ps_sq", tag="st")
            for mc in range(CC):
                ps = pao.tile([128, TQ], F32, name="ps_ao", tag="ao")
                for kc in range(CC):
                    nc.tensor.matmul(ps[:],
                                     wo_sb[kc][:, mc * 128:(mc + 1) * 128],
                                     ctxT_sb[kc][:],
                                     start=(kc == 0), stop=(kc == CC - 1))
                nc.vector.scalar_tensor_tensor(
                    r1_sb[mc][:], ps[:], prm(mc, P_BO), xtq_sb[mc][:],
                    mybir.AluOpType.add, mybir.AluOpType.add)
                nc.tensor.matmul(ps_sum[:], ones_sb[:, 0:1], r1_sb[mc][:],
                                 start=(mc == 0), stop=(mc == CC - 1))
                sq = cpool.tile([128, TQ], F32R, name="sq", tag="sq")
                nc.scalar.activation(sq[:], r1_sb[mc][:], ACTF.Square)
                nc.tensor.matmul(ps_sq[:], ones_sb[:, 0:1], sq[:],
                                 start=(mc == 0), stop=(mc == CC - 1))
            mean1, ex21 = _ln_stats(nc, cpool, ps_sum, ps_sq, "l1")
        xtq_pool.release()
        hT_pool = tc.alloc_tile_pool(name="hTp", bufs=1, side="left")
        hT_sb = [hT_pool.tile([128, TQ], F32R, name=f"hT{cc}")
                 for cc in range(CC)]
        hT_holder["pool"] = hT_pool
        hT_holder["tiles"] = hT_sb
        with tc.tile_pool(name="pbc2", bufs=2, space="PSUM") as pbc2:
            _ln_apply(nc, tc, cpool, pbc2, ones_sb, mean1, ex21, r1_sb, hT_sb,
                      [prm(cc, P_L1S) for cc in range(CC)],
                      [prm(cc, P_L1B) for cc in range(CC)], "l1")

    if taps:
        _tap(nc, tc, taps["hT"], [t[:] for t in hT_sb], TQ)

    # ==================== phase D: MLP + residual + LN2 =================
    with tc.tile_pool(name="dpool", bufs=3, side="right") as dpool, \
         tc.tile_pool(name="w2pool", bufs=3, side="right") as w2pool, \
         tc.tile_pool(name="r2pool", bufs=1, side="right") as r2pool:

        r2_sb = [r2pool.tile([128, TQ], F32R, name=f"r2{cc}")
                 for cc in range(CC)]
        with tc.tile_pool(name="pfc2", bufs=1, space="PSUM") as pfc2:
            ps_m = [pfc2.tile([128, TQ], F32, name=f"ps_m{mc}", tag=f"m{mc}")
                    for mc in range(CC)]
            with tc.tile_pool(name="pfc1", bufs=2, space="PSUM") as pfc1:
                w1blk = {}
                for kc2 in range(ICN):
                    jb = kc2 // CC
                    if kc2 % CC == 0:
                        w1blk[jb] = []
                        for kc in range(CC):
                            t = w1pool.tile([128, C], F32R, name="w1_t",
                                            tag="w1")
                            nc.sync.dma_start(
                                out=t[:],
                                in_=d["w1"][kc * 128:(kc + 1) * 128,
                                            jb * C:(jb + 1) * C])
                            w1blk[jb].append(t)
                    w2t = w2pool.tile([128, C], F32R, name="w2_t", tag="w2")
                    nc.sync.dma_start(
                        out=w2t[:], in_=d["w2"][kc2 * 128:(kc2 + 1) * 128, :])
                    ps1 = pfc1.tile([128, TQ], F32, name="ps1", tag="f1")
                    co = (kc2 % CC) * 128
                    for kc in range(CC):
                        nc.tensor.matmul(
                            ps1[:], w1blk[jb][kc][:, co:co + 128],
                            hT_sb[kc][:],
                            start=(kc == 0), stop=(kc == CC - 1))
                    g = dpool.tile([128, TQ], F32R, name="g", tag="g")
                    nc.scalar.activation(g[:], ps1[:], ACTF.Gelu_apprx_tanh,
                                         bias=b1p_sb[:, kc2].unsqueeze(-1))
                    for mc in range(CC):
                        nc.tensor.matmul(ps_m[mc][:],
                                         w2t[:, mc * 128:(mc + 1) * 128],
                                         g[:], start=(kc2 == 0),
                                         stop=(kc2 == ICN - 1))
            with tc.tile_pool(name="pst2", bufs=2, space="PSUM") as pst2:
                ps_sum2 = pst2.tile([1, TQ], F32, name="ps_sum2", tag="st")
                ps_sq2 = pst2.tile([1, TQ], F32, name="ps_sq2", tag="st")
                for mc in range(CC):
                    nc.vector.scalar_tensor_tensor(
                        r2_sb[mc][:], ps_m[mc][:], prm(mc, P_B2),
                        hT_sb[mc][:], mybir.AluOpType.add,
                        mybir.AluOpType.add)
                    nc.tensor.matmul(ps_sum2[:], ones_sb[:, 0:1], r2_sb[mc][:],
                                     start=(mc == 0), stop=(mc == CC - 1))
                    sq = dpool.tile([128, TQ], F32R, name="sq2", tag="sq")
                    nc.scalar.activation(sq[:], r2_sb[mc][:], ACTF.Square)
                    nc.tensor.matmul(ps_sq2[:], ones_sb[:, 0:1], sq[:],
                                     start=(mc == 0), stop=(mc == CC - 1))
                mean2, ex22 = _ln_stats(nc, dpool, ps_sum2, ps_sq2, "l2")
        hT_pool.release()
        with tc.tile_pool(name="pbc3", bufs=2, space="PSUM") as pbc3:
            outT_sb = [dpool.tile([128, TQ], F32, name=f"o{cc}", tag=f"o{cc}",
                                  bufs=1) for cc in range(CC)]
            _ln_apply(nc, tc, dpool, pbc3, ones_sb, mean2, ex22, r2_sb,
                      outT_sb,
                      [prm(cc, P_L2S) for cc in range(CC)],
                      [prm(cc, P_L2B) for cc in range(CC)], "l2")
            for cc in range(CC):
                nc.sync.dma_start(out=d["outT"][cc * 128:(cc + 1) * 128, :],
                                  in_=outT_sb[cc][:])

    w1pool.release()
    ctxT_pool.release()
    const.release()


_NC = None


def _get_nc():
    global _NC
    if _NC is None:
        _NC = _build_nc()
    return _NC


def _prep_inmaps(x, Wq, bq, Wk, bk, Wv, bv, Wo, bo, ln1_s, ln1_b,
                 W1, b1, W2, b2, ln2_s, ln2_b):
    f32 = np.float32
    xT = [np.ascontiguousarray(np.asarray(x)[b].T, dtype=f32)
          for b in range(B)]
    wq = np.ascontiguousarray(Wq, dtype=f32)
    wk = np.ascontiguousarray(Wk, dtype=f32)
    wv = np.ascontiguousarray(Wv, dtype=f32)
    wo = np.ascontiguousarray(Wo, dtype=f32)
    w1 = np.ascontiguousarray(W1, dtype=f32)
    w2 = np.ascontiguousarray(W2, dtype=f32)
    prk = np.zeros((128, CC, 8), f32)
    for pi, arr in ((P_BQ, bq), (P_BK, bk), (P_BO, bo), (P_B2, b2),
                    (P_L1S, ln1_s), (P_L1B, ln1_b), (P_L2S, ln2_s),
                    (P_L2B, ln2_b)):
        prk[:, :, pi] = np.asarray(arr, f32).reshape(CC, 128).T
    b1p = np.ascontiguousarray(np.asarray(b1, f32).reshape(ICN, 128).T)
    bvb = np.broadcast_to(np.asarray(bv, f32)[None, :], (128, C)).copy()
    ones = np.ones((128, 128), f32)
    kk = np.arange(128)[:, None]
    qq = np.arange(TQ)[None, :]
    in_maps = []
    for c in range(8):
        b, p = c // 4, c % 4
        msk = np.zeros((4, 128, 2 * TQ), ml_dtypes.bfloat16)
        for kc2 in range(4):
            sl1 = 64 * kc2 + 32
            m0 = ((128 * (2 * kc2) + kk) <= (p + 4 * qq))
            m1 = ((128 * (2 * kc2 + 1) + kk) <= (p + 4 * qq))
            msk[kc2, :, 0:TQ] = m0.astype(ml_dtypes.bfloat16)
            msk[kc2, :, TQ:2 * TQ - sl1] = m1[:, sl1:].astype(
                ml_dtypes.bfloat16)
        mskq = np.zeros((2, 128, 2 * TQ), ml_dtypes.bfloat16)
        qqh = np.arange(256)[None, :] + 256
        for j in range(2):
            for i in range(4):
                kc = (j + 2) * 4 + i
                mskq[j, :, i * 256:(i + 1) * 256] = (
                    (128 * kc + kk) <= (p + 4 * qqh)).astype(
                        ml_dtypes.bfloat16)
        in_maps.append({
            "xt": xT[b], "xtq": np.ascontiguousarray(xT[b][:, p::4]),
            "wq": wq, "wk": wk, "wv": wv, "wo": wo, "w1": w1, "w2": w2,
            "prk": prk, "b1p": b1p, "bvb": bvb, "msk": msk, "mskq": mskq,
            "ones": ones,
            "zer": np.zeros((DH, TQ), f32),
        })
    return in_maps


def _run(in_maps, trace=False, **kw):
    nc = _get_nc()
    return run_bass_kernel_spmd(nc, in_maps, list(range(8)), trace=trace, **kw)


def kernel(**inputs):
    in_maps = _prep_inmaps(**inputs)
    res = _run(in_maps)
    out = np.empty((B, T, C), np.float32)
    for c in range(8):
        b, p = c // 4, c % 4
        out[b, p::4, :] = res.results[c]["outT"].T
    return out


# revision 34
# speedup vs baseline: 1.0758x; 1.0186x over previous
"""Trainium2 Bass kernel for one transformer block (B=2, T=2048, C=768, H=12,
inner=3072, fp32, causal, post-norm residual).

Sharding: 8 cores, token-interleaved. Core c handles batch c//4, tokens
p::4 (p = c%4) of that batch — every core runs the IDENTICAL program
(SPMD), causality is entirely data-driven via per-core mask tensors.

Layout: all activations transposed [C, tokens] so matmul contractions land
on partitions. K/V are (redundantly) computed for the full 2048 tokens on
every core; attention runs as scores^T -> exp -> [V|ones] matmul which
accumulates ctx^T and the softmax denominator in one pass (no max
subtraction needed: scores are ~N(0,1), |s|max << fp32 exp range).
Partition-dim reductions (softmax sums, layernorm stats) use ones-vector
matmuls on the PE; partition broadcasts use K=1 outer-product matmuls.

Matmuls run in float32r (tf32-like, full PE rate at N>=256); everything
else fp32.
"""

import sys

if "/opt/trn_rl_repo" not in sys.path:
    sys.path.insert(0, "/opt/trn_rl_repo")

import numpy as np
import ml_dtypes

import concourse.bacc as bacc
import concourse.mybir as mybir
import concourse.tile as tile
from concourse.bass_utils import run_bass_kernel_spmd

F32 = mybir.dt.float32
F32R = mybir.dt.float32r
BF16 = mybir.dt.bfloat16
ACTF = mybir.ActivationFunctionType

B, T, C = 2, 2048, 768
H, DH = 12, 64
IN = 3072
CC = C // 128          # 6 channel chunks
TBN = T // 512         # 4 token blocks of full seq
TQ = 512               # tokens per core
KCN = T // 128         # 16 k-chunks
ICN = IN // 128        # 24 inner chunks
HG = 2                 # head groups
HPG = H // HG          # 6 heads per group
EPS = 1e-4
SCALE = 1.0 / np.sqrt(DH)

# param pack order in "prk" [128, CC, 8]
P_BQ, P_BK, P_BO, P_B2, P_L1S, P_L1B, P_L2S, P_L2B = range(8)

DEBUG_TAPS = False


def _build_nc():
    nc = bacc.Bacc("TRN2", target_bir_lowering=False, debug=False,
                   enable_asserts=False, num_devices=8)
    d = {}
    d["xt"] = nc.dram_tensor("xt", [C, T], F32R, kind="ExternalInput").ap()
    d["xtq"] = nc.dram_tensor("xtq", [C, TQ], F32R, kind="ExternalInput").ap()
    for w, sh in (("wq", [C, C]), ("wk", [C, C]), ("wv", [C, C]),
                  ("wo", [C, C]), ("w1", [C, IN]), ("w2", [IN, C])):
        d[w] = nc.dram_tensor(w, sh, F32R, kind="ExternalInput").ap()
    d["prk"] = nc.dram_tensor("prk", [128, CC, 8], F32, kind="ExternalInput").ap()
    d["b1p"] = nc.dram_tensor("b1p", [128, ICN], F32, kind="ExternalInput").ap()
    d["bvb"] = nc.dram_tensor("bvb", [128, C], F32, kind="ExternalInput").ap()
    d["msk"] = nc.dram_tensor("msk", [4, 128, 2 * TQ], BF16, kind="ExternalInput").ap()
    d["mskq"] = nc.dram_tensor("mskq", [2, 128, 2 * TQ], BF16, kind="ExternalInput").ap()
    d["ones"] = nc.dram_tensor("ones", [128, 128], F32R, kind="ExternalInput").ap()
    d["zer"] = nc.dram_tensor("zer", [DH, TQ], F32R, kind="ExternalInput").ap()
    d["outT"] = nc.dram_tensor("outT", [C, TQ], F32, kind="ExternalOutput").ap()
    taps = {}
    if DEBUG_TAPS:
        taps["qT"] = nc.dram_tensor("qT", [C, TQ], F32, kind="ExternalOutput").ap()
        taps["kT"] = nc.dram_tensor("kT", [C, T], F32, kind="ExternalOutput").ap()
        taps["v"] = nc.dram_tensor("v", [T, C], F32, kind="ExternalOutput").ap()
        taps["ctxT"] = nc.dram_tensor("ctxT", [C, TQ], F32, kind="ExternalOutput").ap()
        taps["hT"] = nc.dram_tensor("hT", [C, TQ], F32, kind="ExternalOutput").ap()

    with tile.TileContext(nc) as tc:
        _emit(nc, tc, d, taps)
    nc.finalize()
    return nc


def _tap(nc, tc, dst, tiles, width):
    with tc.tile_pool(name="tapp", bufs=2, side="right") as tp:
        for i, src in enumerate(tiles):
            t = tp.tile([128, width], F32, name="tapt", tag="t")
            nc.vector.tensor_copy(t[:], src)
            nc.sync.dma_start(out=dst[i * 128:(i + 1) * 128, :], in_=t[:])


def _ln_stats(nc, pool, ps_sum, ps_sq, tagp):
    """ACT-copy psum stats rows to SBUF (inside the psum pool's scope)."""
    n = float(C)
    mean = pool.tile([1, TQ], F32R, name="ln_mean", tag=tagp + "mean")
    nc.scalar.activation(mean[:], ps_sum[:], ACTF.Copy, scale=1.0 / n)
    ex2 = pool.tile([1, TQ], F32, name="ln_ex2", tag=tagp + "ex2")
    nc.scalar.activation(ex2[:], ps_sq[:], ACTF.Copy, scale=1.0 / n)
    return mean, ex2


def _ln_apply(nc, tc, pool, pbc, ones_sb, mean, ex2, in_sb, out_sb,
              scales, biases, tagp):
    """out = (in - mean)/sqrt(var_unbiased + eps) * s + b, stats over C."""
    n = float(C)
    m2 = pool.tile([1, TQ], F32, name="ln_m2", tag=tagp + "m2")
    nc.vector.tensor_mul(m2[:], mean[:], mean[:])
    dv = pool.tile([1, TQ], F32, name="ln_d", tag=tagp + "d")
    nc.vector.tensor_sub(dv[:], ex2[:], m2[:])
    eps_sb = pool.tile([1, 1], F32, name="ln_eps", tag=tagp + "eps")
    nc.vector.memset(eps_sb[:], float(EPS))
    std = pool.tile([1, TQ], F32, name="ln_std", tag=tagp + "std")
    nc.scalar.activation(std[:], dv[:], ACTF.Sqrt,
                         scale=n / (n - 1.0), bias=eps_sb[:])
    istd = pool.tile([1, TQ], F32R, name="ln_istd", tag=tagp + "istd")
    with nc.allow_low_precision(reason="f32r matmul operand"):
        nc.vector.reciprocal(istd[:], std[:])
    # broadcast mean and istd across partitions via K=1 matmul
    pmb = pbc.tile([128, TQ], F32, name="ln_pmb", tag="bc")
    nc.tensor.matmul(pmb[:], ones_sb[0:1, :], mean[:], start=True, stop=True)
    mb = pool.tile([128, TQ], F32, name="ln_mb", tag=tagp + "mb")
    nc.scalar.activation(mb[:], pmb[:], ACTF.Copy)
    pib = pbc.tile([128, TQ], F32, name="ln_pib", tag="bc")
    nc.tensor.matmul(pib[:], ones_sb[0:1, :], istd[:], start=True, stop=True)
    ib = pool.tile([128, TQ], F32, name="ln_ib", tag=tagp + "ib")
    nc.scalar.activation(ib[:], pib[:], ACTF.Copy)
    for cc in range(CC):
        t1 = pool.tile([128, TQ], F32, name="ln_t1", tag=tagp + "t1")
        nc.vector.tensor_sub(t1[:], in_sb[cc][:], mb[:])
        t2 = pool.tile([128, TQ], F32, name="ln_t2", tag=tagp + "t2")
        nc.vector.tensor_mul(t2[:], t1[:], ib[:])
        nc.vector.tensor_scalar(out_sb[cc][:], t2[:], scales[cc], biases[cc],
                                mybir.AluOpType.mult, mybir.AluOpType.add)


def _emit(nc, tc, d, taps):
    # ---- persistent constants ------------------------------------------
    const = tc.alloc_tile_pool(name="const", bufs=1, side="left")
    ones_sb = const.tile([128, 128], F32R, name="ones_sb")
    prk_sb = const.tile([128, CC, 8], F32, name="prk_sb")
    b1p_sb = const.tile([128, ICN], F32, name="b1p_sb")
    bvb_sb = const.tile([128, C], F32, name="bvb_sb")

    def prm(cc, pi):
        return prk_sb[:, cc, pi].unsqueeze(-1)  # [128,1]

    xtq_pool = tc.alloc_tile_pool(name="xtq", bufs=1, side="left")
    xtq_sb = []
    for cc in range(CC):
        t = xtq_pool.tile([128, TQ], F32R, name=f"xtq{cc}")
        nc.sync.dma_start(out=t[:], in_=d["xtq"][cc * 128:(cc + 1) * 128, :])
        xtq_sb.append(t)

    nc.sync.dma_start(out=ones_sb[:], in_=d["ones"][:])
    nc.sync.dma_start(out=prk_sb[:], in_=d["prk"][:])
    nc.sync.dma_start(out=b1p_sb[:], in_=d["b1p"][:])
    nc.sync.dma_start(out=bvb_sb[:], in_=d["bvb"][:])

    qT_pool = tc.alloc_tile_pool(name="qTp", bufs=1, side="left")
    qz_sb = [qT_pool.tile([128, TQ], F32R, name=f"qz{h}") for h in range(H)]

    kv_pool = tc.alloc_tile_pool(name="kvp", bufs=1, side="left")
    kT_sb = [kv_pool.tile([128, T], F32R, name=f"kT{cc}") for cc in range(CC)]
    v_sb = [kv_pool.tile([128, H, DH + 1], BF16, name=f"v{tch}")
            for tch in range(KCN)]

    # ==================== phase A: QKV ==================================
    with tc.tile_pool(name="wstr", bufs=13, side="right") as wpool, \
         tc.tile_pool(name="xts", bufs=12, side="right") as xt_pool, \
         tc.tile_pool(name="pqkv", bufs=2, space="PSUM") as pqkv:
        pass

        wq_sb = []
        for cc in range(CC):
            t = wpool.tile([128, C], F32R, name="w_t", tag="w")
            nc.sync.dma_start(out=t[:], in_=d["wq"][cc * 128:(cc + 1) * 128, :])
            wq_sb.append(t)
        wk_sb = []
        for cc in range(CC):
            t = wpool.tile([128, C], F32R, name="w_t", tag="w")
            nc.sync.dma_start(out=t[:], in_=d["wk"][cc * 128:(cc + 1) * 128, :])
            wk_sb.append(t)
        # q^T = Wq^T x_q^T + bq; per-head rows ro..ro+DH of qz, rest zero
        for mc in range(CC):
            ps = pqkv.tile([128, TQ], F32, name="ps_q", tag="pq", bufs=3)
            for kc in range(CC):
                nc.tensor.matmul(ps[:], wq_sb[kc][:, mc * 128:(mc + 1) * 128],
                                 xtq_sb[kc][:],
                                 start=(kc == 0), stop=(kc == CC - 1))
            for half in range(2):
                h = 2 * mc + half
                ro = half * DH
                nc.vector.tensor_scalar_add(
                    qz_sb[h][ro:ro + DH, :], ps[ro:ro + DH, :],
                    prm(mc, P_BQ)[ro:ro + DH, :])

        wv_sb = []
        for cc in range(CC):
            t = wpool.tile([128, C], F32R, name="w_t", tag="w")
            nc.sync.dma_start(out=t[:], in_=d["wv"][cc * 128:(cc + 1) * 128, :])
            wv_sb.append(t)

        for tb in range(TBN):
            xt_blk = []
            for cc in range(CC):
                t = xt_pool.tile([128, 512], F32R, name="xt_t", tag="xt")
                nc.sync.dma_start(
                    out=t[:], in_=d["xt"][cc * 128:(cc + 1) * 128,
                                          tb * 512:(tb + 1) * 512])
                xt_blk.append(t)
            # k^T columns of this block
            for mc in range(CC):
                ps = pqkv.tile([128, 512], F32, name="ps_k", tag="pq",
                               bufs=3)
                for kc in range(CC):
                    nc.tensor.matmul(ps[:],
                                     wk_sb[kc][:, mc * 128:(mc + 1) * 128],
                                     xt_blk[kc][:],
                                     start=(kc == 0), stop=(kc == CC - 1))
                nc.vector.tensor_scalar_add(
                    kT_sb[mc][:, tb * 512:(tb + 1) * 512], ps[:],
                    prm(mc, P_BK))
            # v rows (natural layout), 4 chunks of 128 tokens each
            for tci in range(4):
                tch = tb * 4 + tci
                ps1 = pqkv.tile([128, 512], F32, name="ps_v1", tag="pv1")
                ps2 = pqkv.tile([128, 256], F32, name="ps_v2", tag="pv2")
                for kc in range(CC):
                    xsl = xt_blk[kc][:, tci * 128:(tci + 1) * 128]
                    nc.tensor.matmul(ps1[:], xsl, wv_sb[kc][:, 0:512],
                                     start=(kc == 0), stop=(kc == CC - 1))
                    nc.tensor.matmul(ps2[:], xsl, wv_sb[kc][:, 512:C],
                                     start=(kc == 0), stop=(kc == CC - 1))
                vt = v_sb[tch]
                nc.vector.tensor_add(
                    vt[:, 0:8, 0:DH],
                    ps1[:].rearrange("p (h d) -> p h d", d=DH),
                    bvb_sb[:, 0:512].rearrange("p (h d) -> p h d", d=DH))
                nc.vector.tensor_add(
                    vt[:, 8:H, 0:DH],
                    ps2[:].rearrange("p (h d) -> p h d", d=DH),
                    bvb_sb[:, 512:C].rearrange("p (h d) -> p h d", d=DH))
                nc.vector.tensor_copy(vt[:, :, DH], ones_sb[:, 0:H])

    if taps:
        _tap(nc, tc, taps["qT"], [t[:] for t in qT_sb], TQ)
        _tap(nc, tc, taps["kT"], [t[:] for t in kT_sb], T)
        _tap(nc, tc, taps["v"],
             [v_sb[tch][:, :, 0:DH] for tch in range(KCN)], C)

    # ==================== phase B: attention ============================
    for h in range(H):
        ro = (h % 2) * DH
        nc.sync.dma_start(out=qz_sb[h][(DH - ro):(128 - ro), :],
                          in_=d["zer"][:])
    ctxT_pool = tc.alloc_tile_pool(name="ctxTp", bufs=1, side="right")
    ctxT_sb = [ctxT_pool.tile([128, TQ], F32R, name=f"ctxT{cc}")
               for cc in range(CC)]

    with tc.tile_pool(name="mskp", bufs=1, side="right") as mpool, \
         tc.tile_pool(name="attnp", bufs=6, side="right") as apool, \
         tc.tile_pool(name="pctx", bufs=2, space="PSUM") as pctx, \
         tc.tile_pool(name="psc", bufs=3, space="PSUM") as psc:
        msk_sb = []
        for kc2 in range(4):
            mt = mpool.tile([128, 2 * TQ], BF16, name=f"msk{kc2}")
            nc.sync.dma_start(out=mt[:], in_=d["msk"][kc2, :, :])
            msk_sb.append(mt)
        mskq_sb = []
        for j in range(2):
            mt = mpool.tile([128, 2 * TQ], BF16, name=f"mskq{j}")
            nc.sync.dma_start(out=mt[:], in_=d["mskq"][j, :, :])
            mskq_sb.append(mt)
        dn_sb = mpool.tile([65, 4 * TQ], F32, name="dn_sb")
        rcp_all = mpool.tile([65, 4 * TQ], F32R, name="rcp_all")
        nc.vector.memset(dn_sb[:], 1.0)

        pending = []

        def _norm_ops_for_group(g):
            ops = []
            for q in range(4):
                def _recip(g=g, q=q):
                    c0 = g * TQ + q * (TQ // 4)
                    with nc.allow_low_precision(reason="f32r operand"):
                        nc.vector.reciprocal(rcp_all[:, c0:c0 + TQ // 4],
                                             dn_sb[:, c0:c0 + TQ // 4])
                ops.append(_recip)
            for hh in range(g * 3, g * 3 + 3):
                def _one(hh=hh, g=g):
                    cc2, ro2 = hh // 2, (hh % 2) * DH
                    bp2 = (hh % 3) * 32
                    pb = pctx.tile([DH, TQ], F32, name="pb", tag="ctx")
                    nc.tensor.matmul(
                        pb[:DH, :], ones_sb[bp2:bp2 + 1, 0:DH],
                        rcp_all[bp2:bp2 + 1, g * TQ:(g + 1) * TQ],
                        start=True, stop=True)
                    bc = apool.tile([128, TQ], F32, name="bc", tag="bc")
                    nc.scalar.activation(bc[ro2:ro2 + DH, :], pb[:DH, :],
                                         ACTF.Copy)
                    nc.vector.tensor_mul(ctxT_sb[cc2][ro2:ro2 + DH, :],
                                         ctxT_sb[cc2][ro2:ro2 + DH, :],
                                         bc[ro2:ro2 + DH, :])
                ops.append(_one)
            return ops

        for h in range(H):
            cc, ro = h // 2, (h % 2) * DH
            ctx_ps = pctx.tile([DH + 1, TQ], F32, name="ctx_ps", tag="ctx")
            for kc2 in range(4):
                # causal: chunk kc only reaches queries qq >= 32*kc (uniform
                # across cores). Skip the fully-masked left part; pack the
                # two halves contiguously so exp reads one gap-free region.
                s0 = 64 * kc2
                sl1 = 64 * kc2 + 32
                ps = psc.tile([128, 2 * TQ], F32, name="ps_s", tag="s")
                nc.tensor.matmul(
                    ps[:, s0:TQ],
                    kT_sb[cc][:, (2 * kc2) * 128:(2 * kc2 + 1) * 128],
                    qz_sb[h][:, s0:], start=True, stop=True)
                nc.tensor.matmul(
                    ps[:, TQ:2 * TQ - sl1],
                    kT_sb[cc][:, (2 * kc2 + 1) * 128:(2 * kc2 + 2) * 128],
                    qz_sb[h][:, sl1:], start=True, stop=True)
                et = apool.tile([128, 2 * TQ], BF16, name="et", tag="e")
                nc.scalar.activation(et[:, s0:2 * TQ - sl1],
                                     ps[:, s0:2 * TQ - sl1], ACTF.Exp,
                                     scale=float(SCALE))
                nc.vector.tensor_mul(et[:, s0:2 * TQ - sl1],
                                     et[:, s0:2 * TQ - sl1],
                                     msk_sb[kc2][:, s0:2 * TQ - sl1])
                for half in range(2):
                    kc = kc2 * 2 + half
                    qoff = 32 * kc
                    if half == 0:
                        rsl = slice(qoff, TQ)
                    else:
                        rsl = slice(TQ + qoff - sl1, 2 * TQ - sl1)
                    nc.tensor.matmul(ctx_ps[:, qoff:], v_sb[kc][:, h, :],
                                     et[:, rsl],
                                     start=(kc == 0), stop=False)
                if pending:
                    pending.pop(0)()
            for qd in range(2, 4):
                # chunks kc >= 8: each contributes 256 score columns
                # (f32r floor); pack four chunks into one [128,1024] tile
                # so a single exp covers them with no per-call overhead x4
                ps = psc.tile([128, 2 * TQ], F32, name="ps_s", tag="s")
                for i in range(4):
                    kc = qd * 4 + i
                    nc.tensor.matmul(
                        ps[:, i * 256:(i + 1) * 256],
                        kT_sb[cc][:, kc * 128:(kc + 1) * 128],
                        qz_sb[h][:, 256:], start=True, stop=True)
                et = apool.tile([128, 2 * TQ], BF16, name="et", tag="e")
                nc.scalar.activation(et[:], ps[:], ACTF.Exp,
                                     scale=float(SCALE))
                nc.vector.tensor_mul(et[:], et[:], mskq_sb[qd - 2][:])
                for i in range(4):
                    kc = qd * 4 + i
                    qoff = 32 * kc
                    rsl = slice(i * 256 + qoff - 256, (i + 1) * 256)
                    nc.tensor.matmul(ctx_ps[:, qoff:], v_sb[kc][:, h, :],
                                     et[:, rsl],
                                     start=False, stop=(kc == KCN - 1))
                if pending:
                    pending.pop(0)()
            # evict unnormalized ctx + denominator row; normalize later
            nc.scalar.activation(ctxT_sb[cc][ro:ro + DH, :],
                                 ctx_ps[0:DH, :], ACTF.Copy)
            bp, g = (h % 3) * 32, h // 3
            nc.vector.tensor_copy(dn_sb[bp:bp + 1, g * TQ:(g + 1) * TQ],
                                  ctx_ps[DH:DH + 1, :])
            if h % 3 == 2:
                pending.extend(_norm_ops_for_group(h // 3))
        for op in pending:
            op()

    kv_pool.release()
    qT_pool.release()

    if taps:
        _tap(nc, tc, taps["ctxT"], [t[:] for t in ctxT_sb], TQ)

    # ==================== phase C: Wo + residual + LN1 ==================
    w1pool = tc.alloc_tile_pool(name="w1pool", bufs=2 * CC, side="right")
    hT_holder = {}

    with tc.tile_pool(name="cpool", bufs=2, side="right") as cpool, \
         tc.tile_pool(name="wopool", bufs=7, side="right") as wopool, \
         tc.tile_pool(name="r1pool", bufs=1, side="right") as r1pool:
        wo_sb = []
        for cc in range(CC):
            t = wopool.tile([128, C], F32R, name="wo_t", tag="wo")
            nc.sync.dma_start(out=t[:], in_=d["wo"][cc * 128:(cc + 1) * 128, :])
            wo_sb.append(t)
        r1_sb = [r1pool.tile([128, TQ], F32R, name=f"r1{cc}")
                 for cc in range(CC)]
        with tc.tile_pool(name="pao", bufs=2, space="PSUM") as pao, \
             tc.tile_pool(name="pst", bufs=2, space="PSUM") as pst:
            ps_sum = pst.tile([1, TQ], F32, name="ps_sum", tag="st")
            ps_sq = pst.tile([1, TQ], F32, name="ps_sq", tag="st")
            for mc in range(CC):
                ps = pao.tile([128, TQ], F32, name="ps_ao", tag="ao")
                for kc in range(CC):
                    nc.tensor.matmul(ps[:],
                                     wo_sb[kc][:, mc * 128:(mc + 1) * 128],
                                     ctxT_sb[kc][:],
                                     start=(kc == 0), stop=(kc == CC - 1))
                nc.vector.scalar_tensor_tensor(
                    r1_sb[mc][:], ps[:], prm(mc, P_BO), xtq_sb[mc][:],
                    mybir.AluOpType.add, mybir.AluOpType.add)
                nc.tensor.matmul(ps_sum[:], ones_sb[:, 0:1], r1_sb[mc][:],
                                 start=(mc == 0), stop=(mc == CC - 1))
                sq = cpool.tile([128, TQ], F32R, name="sq", tag="sq")
                nc.scalar.activation(sq[:], r1_sb[mc][:], ACTF.Square)
                nc.tensor.matmul(ps_sq[:], ones_sb[:, 0:1], sq[:],
                                 start=(mc == 0), stop=(mc == CC - 1))
            mean1, ex21 = _ln_stats(nc, cpool, ps_sum, ps_sq, "l1")
        xtq_pool.release()
        hT_pool = tc.alloc_tile_pool(name="hTp", bufs=1, side="left")
        hT_sb = [hT_pool.tile([128, TQ], F32R, name=f"hT{cc}")
                 for cc in range(CC)]
        hT_holder["pool"] = hT_pool
        hT_holder["tiles"] = hT_sb
        with tc.tile_pool(name="pbc2", bufs=2, space="PSUM") as pbc2:
            _ln_apply(nc, tc, cpool, pbc2, ones_sb, mean1, ex21, r1_sb, hT_sb,
                      [prm(cc, P_L1S) for cc in range(CC)],
                      [prm(cc, P_L1B) for cc in range(CC)], "l1")

    if taps:
        _tap(nc, tc, taps["hT"], [t[:] for t in hT_sb], TQ)

    # ==================== phase D: MLP + residual + LN2 =================
    with tc.tile_pool(name="dpool", bufs=3, side="right") as dpool, \
         tc.tile_pool(name="w2pool", bufs=3, side="right") as w2pool, \
         tc.tile_pool(name="r2pool", bufs=1, side="right") as r2pool:

        r2_sb = [r2pool.tile([128, TQ], F32R, name=f"r2{cc}")
                 for cc in range(CC)]
        with tc.tile_pool(name="pfc2", bufs=1, space="PSUM") as pfc2:
            ps_m = [pfc2.tile([128, TQ], F32, name=f"ps_m{mc}", tag=f"m{mc}")
                    for mc in range(CC)]
            with tc.tile_pool(name="pfc1", bufs=2, space="PSUM") as pfc1:
                w1blk = {}
                for kc2 in range(ICN):
                    jb = kc2 // CC
                    if kc2 % CC == 0:
                        w1blk[jb] = []
                        for kc in range(CC):
                            t = w1pool.tile([128, C], F32R, name="w1_t",
                                            tag="w1")
                            nc.sync.dma_start(
                                out=t[:],
                                in_=d["w1"][kc * 128:(kc + 1) * 128,
                                            jb * C:(jb + 1) * C])
                            w1blk[jb].append(t)
                    w2t = w2pool.tile([128, C], F32R, name="w2_t", tag="w2")
                    nc.sync.dma_start(
                        out=w2t[:], in_=d["w2"][kc2 * 128:(kc2 + 1) * 128, :])
                    ps1 = pfc1.tile([128, TQ], F32, name="ps1", tag="f1")
                    co = (kc2 % CC) * 128
                    for kc in range(CC):
                        nc.tensor.matmul(
                            ps1[:], w1blk[jb][kc][:, co:co + 128],
                            hT_sb[kc][:],
                            start=(kc == 0), stop=(kc == CC - 1))
                    g = dpool.tile([128, TQ], F32R, name="g", tag="g")
                    nc.scalar.activation(g[:], ps1[:], ACTF.Gelu_apprx_tanh,
                                         bias=b1p_sb[:, kc2].unsqueeze(-1))
                    for mc in range(CC):
                        nc.tensor.matmul(ps_m[mc][:],
                                         w2t[:, mc * 128:(mc + 1) * 128],
                                         g[:], start=(kc2 == 0),
                                         stop=(kc2 == ICN - 1))
            with tc.tile_pool(name="pst2", bufs=2, space="PSUM") as pst2:
                ps_sum2 = pst2.tile([1, TQ], F32, name="ps_sum2", tag="st")
                ps_sq2 = pst2.tile([1, TQ], F32, name="ps_sq2", tag="st")
                for mc in range(CC):
                    nc.vector.scalar_tensor_tensor(
                        r2_sb[mc][:], ps_m[mc][:], prm(mc, P_B2),
                        hT_sb[mc][:], mybir.AluOpType.add,
                        mybir.AluOpType.add)
                    nc.tensor.matmul(ps_sum2[:], ones_sb[:, 0:1], r2_sb[mc][:],
                                     start=(mc == 0), stop=(mc == CC - 1))
                    sq = dpool.tile([128, TQ], F32R, name="sq2", tag="sq")
                    nc.scalar.activation(sq[:], r2_sb[mc][:], ACTF.Square)
                    nc.tensor.matmul(ps_sq2[:], ones_sb[:, 0:1], sq[:],
                                     start=(mc == 0), stop=(mc == CC - 1))
                mean2, ex22 = _ln_stats(nc, dpool, ps_sum2, ps_sq2, "l2")
        hT_pool.release()
        with tc.tile_pool(name="pbc3", bufs=2, space="PSUM") as pbc3:
            outT_sb = [dpool.tile([128, TQ], F32, name=f"o{cc}", tag=f"o{cc}",
                                  bufs=1) for cc in range(CC)]
            _ln_apply(nc, tc, dpool, pbc3, ones_sb, mean2, ex22, r2_sb,
                      outT_sb,
                      [prm(cc, P_L2S) for cc in range(CC)],
                      [prm(cc, P_L2B) for cc in range(CC)], "l2")
            for cc in range(CC):
                nc.sync.dma_start(out=d["outT"][cc * 128:(cc + 1) * 128, :],
                                  in_=outT_sb[cc][:])

    w1pool.release()
    ctxT_pool.release()
    const.release()


_NC = None


def _get_nc():
    global _NC
    if _NC is None:
        _NC = _build_nc()
    return _NC


def _prep_inmaps(x, Wq, bq, Wk, bk, Wv, bv, Wo, bo, ln1_s, ln1_b,
                 W1, b1, W2, b2, ln2_s, ln2_b):
    f32 = np.float32
    xT = [np.ascontiguousarray(np.asarray(x)[b].T, dtype=f32)
          for b in range(B)]
    wq = np.ascontiguousarray(Wq, dtype=f32)
    wk = np.ascontiguousarray(Wk, dtype=f32)
    wv = np.ascontiguousarray(Wv, dtype=f32)
    wo = np.ascontiguousarray(Wo, dtype=f32)
    w1 = np.ascontiguousarray(W1, dtype=f32)
    w2 = np.ascontiguousarray(W2, dtype=f32)
    prk = np.zeros((128, CC, 8), f32)
    for pi, arr in ((P_BQ, bq), (P_BK, bk), (P_BO, bo), (P_B2, b2),
                    (P_L1S, ln1_s), (P_L1B, ln1_b), (P_L2S, ln2_s),
                    (P_L2B, ln2_b)):
        prk[:, :, pi] = np.asarray(arr, f32).reshape(CC, 128).T
    b1p = np.ascontiguousarray(np.asarray(b1, f32).reshape(ICN, 128).T)
    bvb = np.broadcast_to(np.asarray(bv, f32)[None, :], (128, C)).copy()
    ones = np.ones((128, 128), f32)
    kk = np.arange(128)[:, None]
    qq = np.arange(TQ)[None, :]
    in_maps = []
    for c in range(8):
        b, p = c // 4, c % 4
        msk = np.zeros((4, 128, 2 * TQ), ml_dtypes.bfloat16)
        for kc2 in range(4):
            sl1 = 64 * kc2 + 32
            m0 = ((128 * (2 * kc2) + kk) <= (p + 4 * qq))
            m1 = ((128 * (2 * kc2 + 1) + kk) <= (p + 4 * qq))
            msk[kc2, :, 0:TQ] = m0.astype(ml_dtypes.bfloat16)
            msk[kc2, :, TQ:2 * TQ - sl1] = m1[:, sl1:].astype(
                ml_dtypes.bfloat16)
        mskq = np.zeros((2, 128, 2 * TQ), ml_dtypes.bfloat16)
        qqh = np.arange(256)[None, :] + 256
        for j in range(2):
            for i in range(4):
                kc = (j + 2) * 4 + i
                mskq[j, :, i * 256:(i + 1) * 256] = (
                    (128 * kc + kk) <= (p + 4 * qqh)).astype(
                        ml_dtypes.bfloat16)
        in_maps.append({
            "xt": xT[b], "xtq": np.ascontiguousarray(xT[b][:, p::4]),
            "wq": wq, "wk": wk, "wv": wv, "wo": wo, "w1": w1, "w2": w2,
            "prk": prk, "b1p": b1p, "bvb": bvb, "msk": msk, "mskq": mskq,
            "ones": ones,
            "zer": np.zeros((DH, TQ), f32),
        })
    return in_maps


def _run(in_maps, trace=False, **kw):
    nc = _get_nc()
    return run_bass_kernel_spmd(nc, in_maps, list(range(8)), trace=trace, **kw)


def kernel(**inputs):
    in_maps = _prep_inmaps(**inputs)
    res = _run(in_maps)
    out = np.empty((B, T, C), np.float32)
    for c in range(8):
        b, p = c // 4, c % 4
        out[b, p::4, :] = res.results[c]["outT"].T
    return out
